# revision 8
# baseline (speedup 1.0000x reference)
"""AscendRejectionSampler — Trainium2 Bass kernel (8-core SPMD), v3.5.

Work split (device does all O(rows x V) arithmetic, host does O(N)
scalar logic + gathers + dtype packing + final assembly):

- Host computes the accept/reject chain (dp/tp/u are O(N) gathers, the
  same class as the uniform_probs/draft-token packing v2 already did)
  -> first_rej per request, so each non-greedy request needs exactly
  ONE (t, d) row on device: argmax((t-d)/q).
- Greedy requests need row-max of each target row (mismatch test
  happens on host by comparing fp16(tp) vs the returned row-max) plus
  the argmax *position* per row.
- Device inputs are fp16 (t, d rows) / bf16 (q rows): ~2.75 MB per
  core instead of 5.7 MB.  Device returns a [128, 5] f32 tile of
  per-partition (max, argmax-window, window-pos) data; host decodes
  and assembles the [64, 9] output.

Device program per core (4 greedy + 4 non-greedy requests):
- tg [128, 8000] f16: 32 greedy target rows x 4 quarters, loaded as 5
  uneven chunks [1000,2000,2000,2000,1000] alternating the two HWDGE
  rings (small first chunk starts DVE early, small last chunk keeps
  the final fold+reduce short; 2-4KB lines are the DMA sweet spot).
- per chunk: one pairwise-max fold (fp16 tensor_tensor runs 2
  elem/cycle) + 125-wide tensor_reduce -> dense cmax [128, 32].
- max8/max_index on cmax -> winning window ci; ONE indirect gather of
  row p*32+ci from tgp, a host-side pair-interleaved copy of tg where
  each window's two fold halves are a contiguous 250-elem row;
  max_index on the window gives the position.  (tgp costs DRAM
  footprint but is only ever read 250 elems/partition.)
- ng [128, 2000] f16 (t|d of the 4 winning rows, 32 segs each) and
  qn [128, 1000] bf16: rq = Exp(-Ln(q)) runs on the otherwise-idle
  ACT engine (Reciprocal table is gated off), diff and ratio are fp16
  2x tensor_tensor ops, max8 + max_index -> seg max and position.
- gpsimd tensor ops are avoided entirely: they stall DVE ~4x via SBUF
  port contention (measured).  tensor_tensor_reduce crashes this
  runtime (unsupported) — plain ops only.
"""
import sys
sys.path.insert(0, '/opt/trn_rl_repo')
import numpy as np
import concourse.bass as bass
import concourse.bacc as bacc
import concourse.tile as tile
from concourse import mybir
from concourse import bass_utils
from concourse.tile import add_dep_helper

f32 = mybir.dt.float32
i32 = mybir.dt.int32
u32 = mybir.dt.uint32
f16 = mybir.dt.float16
bf16 = mybir.dt.bfloat16
Alu = mybir.AluOpType
AX = mybir.AxisListType

R = 8             # requests per core
S = 8
SP1 = 9
V = 32000
B = 64
NG = 4            # slots 0-3 non-greedy, 4-7 greedy
QW = 8000         # quarter width (vocab span per greedy partition)
CW = 2000         # chunk width (per greedy partition, 4 chunks)
SEGW = 1000       # ng segment width (32 segs per row)
WIN = 125         # window width
F16_NINF = -65504.0
CHUNK_WS = [1000, 1500, 2000, 2000, 1500]
CHUNK_BASES = [0, 1000, 2500, 4500, 6500]
# window ci -> (chunk base, within-chunk 125-block, fold-partner stride)
_WINMAP = []
for _b, _w in zip(CHUNK_BASES, CHUNK_WS):
    for _c in range(_w // 250):
        _WINMAP.append((_b, _c, _w // 2))


def build_kernel_fast(n_devices=8):
    nc = bacc.Bacc("TRN2", target_bir_lowering=False, debug=False,
                   enable_asserts=False, num_devices=n_devices)
    tg = nc.dram_tensor("tg", [128, QW], f16, kind="ExternalInput").ap()
    tgp = nc.dram_tensor("tgp", [128, 32 * 256], f16,
                         kind="ExternalInput").ap()
    ngt = nc.dram_tensor("ngt", [128, 2 * SEGW], f16,
                         kind="ExternalInput").ap()
    qn = nc.dram_tensor("qn", [128, SEGW], bf16, kind="ExternalInput").ap()
    out = nc.dram_tensor("out", [128, 5], f32, kind="ExternalOutput").ap()
    with tile.TileContext(nc) as tc:
        _body(tc, nc, tg, tgp, ngt, qn, out)
    nc.compile()
    return nc


def _body(tc, nc, tg, tgp, ngt, qn, out):
    import contextlib
    ctx = contextlib.ExitStack()
    with ctx:
        small = ctx.enter_context(tc.tile_pool(name="small", bufs=1))
        big = ctx.enter_context(tc.tile_pool(name="big", bufs=1))

        V_ = nc.vector
        G_ = nc.gpsimd
        A_ = nc.scalar

        def tt(out_, a, b, op):
            return V_.tensor_tensor(out_, a, b, op=op)

        # ---------------- DMAs: greedy chunks first, uneven widths --
        # small first chunk starts DVE early, small last chunk keeps the
        # final fold+reduce short; ng/q data behind them.
        # widths in elements; each W must be a multiple of 250.
        WS = [1000, 1500, 2000, 2000, 1500]
        BASES = [0, 1000, 2500, 4500, 6500]
        NCH = len(WS)
        ch = [big.tile([128, WS[k]], f16, name=f"tg{k}")
              for k in range(NCH)]
        qtile = big.tile([128, SEGW], bf16)
        ngtile = big.tile([128, 2 * SEGW], f16)
        ring = [nc.sync, nc.scalar, nc.sync, nc.scalar, nc.sync]
        for k in range(NCH):
            ring[k].dma_start(ch[k][:], tg[:, BASES[k]:BASES[k] + WS[k]])
        # q right after c3 on the scalar ring: early enough that the
        # ACT Ln/Exp reciprocal beats the ng multiply, late enough not
        # to displace any greedy chunk
        nc.scalar.dma_start(qtile[:], qn[:, :])
        nc.scalar.dma_start(ngtile[:], ngt[:, :])

        # gpsimd: partition iota (p*32) for the window-gather offsets
        iota32 = small.tile([128, 1], i32)
        G_.iota(iota32[:], pattern=[[0, 1]], base=0, channel_multiplier=32)

        # ---------------- greedy: per-chunk fp16 pairwise-max fold (2x)
        # + 125-wide reduce into a dense cmax[128,32]; chunk k's W/250
        # entries cover x{base_k + 125c + (W_k/2)m}, m in {0,1}.
        cmax = small.tile([128, 32], f16)
        l1 = big.tile([128, QW // 2], f16)
        prev = None
        lb = 0
        cb = 0
        for k in range(NCH):
            h = WS[k] // 2
            fi = tt(l1[:, lb:lb + h],
                    ch[k][:, 0:h], ch[k][:, h:2 * h], Alu.max)
            if prev is not None:
                add_dep_helper(fi.ins, prev.ins, sync=False,
                               reason="DVE: consume chunks in arrival order")
            nwin = WS[k] // 250
            prev = V_.tensor_reduce(
                cmax[:, cb:cb + nwin],
                l1[:, lb:lb + h].rearrange("p (b c) -> p b c", c=WIN),
                axis=AX.X, op=Alu.max)
            lb += h
            cb += nwin

        cm8 = small.tile([128, 8], f16)
        V_.max(out=cm8[:], in_=cmax[:])
        ci8 = small.tile([128, 8], u32)
        V_.max_index(out=ci8[:], in_max=cm8[:], in_values=cmax[:])

        # ONE window gather: tgp is the host-side 125-interleaved pair
        # layout, where window ci's two fold halves are the contiguous
        # 250-wide row p*32 + ci of the [128*32, 250] view.
        offs = small.tile([128, 1], i32)
        o2i = tt(offs[:], iota32[:], ci8.bitcast(i32)[:, 0:1], Alu.add)
        win = big.tile([128, 256], f16)
        tgv = tgp.rearrange("p (g w) -> (p g) w", w=256)
        G_.indirect_dma_start(
            out=win[:], out_offset=None, in_=tgv,
            in_offset=bass.IndirectOffsetOnAxis(ap=offs[:], axis=0))
        wi8 = small.tile([128, 8], u32)
        V_.max_index(out=wi8[:], in_max=cm8[:], in_values=win[:])

        # ---------------- non-greedy: ratio = (t - d) * (1/q), argmax.
        # 1/q = Exp(-Ln(q)) runs entirely on the otherwise-idle ACT
        # engine (the Reciprocal table is gated off); bf16 rq keeps the
        # ratio multiply in the fp16 2x DVE mode.
        lq = big.tile([128, SEGW], f32)
        A_.activation(lq[:], qtile[:], mybir.ActivationFunctionType.Ln)
        rq = big.tile([128, SEGW], bf16)
        A_.activation(rq[:], lq[:], mybir.ActivationFunctionType.Exp,
                      scale=-1.0)
        diff = big.tile([128, SEGW], f16)
        di = tt(diff[:], ngtile[:, 0:SEGW], ngtile[:, SEGW:2 * SEGW],
                Alu.subtract)
        # keep the gather-offset chain ahead of the ng chain on DVE
        add_dep_helper(di.ins, o2i.ins, sync=False,
                       reason="DVE: window gathers in flight before ng work")
        ratio = big.tile([128, SEGW], f16)
        tt(ratio[:], diff[:], rq[:], Alu.mult)
        # two fp16 2x folds: scan only [128,250]; the 4-way position
        # ambiguity is resolved host-side with the exact f32 formula
        rf1 = big.tile([128, SEGW // 2], f16)
        tt(rf1[:], ratio[:, 0:500], ratio[:, 500:1000], Alu.max)
        rf2 = big.tile([128, SEGW // 4], f16)
        tt(rf2[:], rf1[:, 0:250], rf1[:, 250:500], Alu.max)
        n8 = small.tile([128, 8], f16)
        V_.max(out=n8[:], in_=rf2[:])
        ni8 = small.tile([128, 8], u32)
        V_.max_index(out=ni8[:], in_max=n8[:], in_values=rf2[:])

        # ---------------- output pack [128,5]: qmax, ci, wpos, segmax, npos
        # (DVE casts -- keeping ACT on the Ln/Exp table set)
        outp = small.tile([128, 5], f32)
        V_.tensor_copy(out=outp[:, 0:1], in_=cm8[:, 0:1])
        V_.tensor_copy(out=outp[:, 1:2], in_=ci8[:, 0:1])
        V_.tensor_copy(out=outp[:, 2:3], in_=wi8[:, 0:1])
        V_.tensor_copy(out=outp[:, 3:4], in_=n8[:, 0:1])
        V_.tensor_copy(out=outp[:, 4:5], in_=ni8[:, 0:1])
        nc.sync.dma_start(out[:, :], outp[:], single_packet=True)


# ---------------- host side ----------------

def plan_permutation(inputs, n_cores=8):
    """Slot assignment: 4 non-greedy (slots 0-3) + 4 greedy (slots 4-7)."""
    isg = np.asarray(inputs["is_greedy"]).astype(bool)
    Bb = isg.shape[0]
    if Bb != B or Bb // n_cores != R:
        return None
    g = np.where(isg)[0]
    n = np.where(~isg)[0]
    if len(g) != Bb // 2:
        return None
    perm = np.empty(Bb, np.int64)
    for c in range(n_cores):
        perm[c * R:c * R + NG] = n[c * NG:(c + 1) * NG]
        perm[c * R + NG:(c + 1) * R] = g[c * NG:(c + 1) * NG]
    return perm


class _HostCtx:
    pass


def host_precompute(inputs, perm):
    """The O(N) scalar chain: accept/reject, first_rej, winning rows."""
    hc = _HostCtx()
    cu = inputs["cu_num_draft_tokens"].astype(np.int64)
    N = inputs["draft_token_ids"].shape[0]
    n_per = np.diff(np.concatenate([[0], cu]))
    start = cu - n_per
    gidx = np.clip(start[:, None] + np.arange(S)[None, :], 0, N - 1)  # [B,S]
    dtok = inputs["draft_token_ids"][gidx].astype(np.int64)           # [B,S]
    dp = inputs["draft_probs"][gidx, dtok]                            # [B,S]
    tp = inputs["target_probs"][gidx, dtok]
    u = inputs["uniform_probs"][gidx]
    jj = np.arange(S)[None, :]
    valid = jj < n_per[:, None]
    acc = (dp > 0.0) & ((tp / dp) >= u)
    rej = (~acc) & valid
    first_rej = np.where(rej.any(axis=1), rej.argmax(axis=1), S)      # [B]
    win_j = np.minimum(first_rej, np.maximum(n_per - 1, 0))
    win_row = gidx[np.arange(B), win_j]                               # [B]
    hc.n_per, hc.gidx, hc.dtok, hc.tp = n_per, gidx, dtok, tp
    hc.first_rej, hc.win_row, hc.perm = first_rej, win_row, perm
    return hc


def shard_inputs_fast(inputs, perm, n_cores=8, hc=None):
    import ml_dtypes
    if hc is None:
        hc = host_precompute(inputs, perm)
    tprobs = inputs["target_probs"]
    dprobs = inputs["draft_probs"]
    qarr = inputs["q"]
    in_maps = []
    for c in range(n_cores):
        reqs = perm[c * R:(c + 1) * R]
        # non-greedy slots 0-3: one winning row each, 32 segs
        ngp = np.empty((128, 2 * SEGW), np.float16)
        qp = np.empty((128, SEGW), ml_dtypes.bfloat16)
        for s in range(NG):
            req = reqs[s]
            w = hc.win_row[req]
            ngp[s * 32:(s + 1) * 32, 0:SEGW] = \
                tprobs[w].reshape(32, SEGW).astype(np.float16)
            ngp[s * 32:(s + 1) * 32, SEGW:] = \
                dprobs[w].reshape(32, SEGW).astype(np.float16)
            qp[s * 32:(s + 1) * 32] = \
                qarr[req].reshape(32, SEGW).astype(ml_dtypes.bfloat16)
        # greedy slots 4-7: 8 target rows each, 4 quarters per row
        tgq = np.empty((128, QW), np.float16)
        for t in range(NG):
            req = reqs[NG + t]
            rows = hc.gidx[req]
            tgq[t * 32:(t + 1) * 32] = \
                tprobs[rows].astype(np.float16).reshape(32, QW)
        # pair-interleaved copy: window ci's two fold halves contiguous
        tgpair = np.empty_like(tgq)
        for b, w in zip(CHUNK_BASES, CHUNK_WS):
            blk = tgq[:, b:b + w].reshape(128, 2, w // 250, WIN)
            tgpair[:, b:b + w] = np.ascontiguousarray(
                blk.transpose(0, 2, 1, 3)).reshape(128, w)
        # rows padded 250 -> 256 elems (512B gather descriptors)
        tgpad = np.zeros((128, 32, 256), np.float16)
        tgpad[:, :, 0:250] = tgpair.reshape(128, 32, 250)
        in_maps.append(dict(tg=tgq, tgp=tgpad.reshape(128, 32 * 256),
                            ngt=ngp, qn=qp))
    return in_maps


def assemble_outputs_fast(results, perm, inputs, hc):
    out = np.array(inputs["output_token_ids"], dtype=np.int32).copy()
    bonus = inputs["bonus_token_ids"]
    for c in range(len(results)):
        o = np.asarray(results[c]["out"], dtype=np.float64)  # [128,5]
        reqs = perm[c * R:(c + 1) * R]
        for s in range(NG):                    # non-greedy
            req = reqs[s]
            npr = int(hc.n_per[req])
            fr = int(hc.first_rej[req])
            row = out[req]
            for j in range(min(fr, npr)):
                row[j] = hc.dtok[req, j]
            if fr < npr:
                segs = o[s * 32:(s + 1) * 32]
                vals = segs[:, 3]
                seg = int(vals.argmax())       # first max (vocab order)
                ni = int(segs[seg, 4])
                w = hc.win_row[req]
                best, bestv = 0, -np.inf
                for m in range(4):
                    pos = seg * SEGW + ni + 250 * m
                    pr = max(
                        float(inputs["target_probs"][w, pos])
                        - float(inputs["draft_probs"][w, pos]), 0.0
                    ) / float(inputs["q"][req, pos])
                    if pr > bestv:
                        bestv, best = pr, pos
                row[fr] = best
            elif npr < SP1:
                row[npr] = bonus[req]
        for t in range(NG):                    # greedy
            req = reqs[NG + t]
            npr = int(hc.n_per[req])
            tok = np.empty(S, np.int64)
            mism = np.zeros(S, bool)
            for j in range(npr):
                q4 = o[(t * 8 + j) * 4:(t * 8 + j) * 4 + 4]
                qmax = q4[:, 0]
                qt = int(qmax.argmax())        # first quarter with max
                rowmax = qmax[qt]
                tp16 = np.float32(np.float16(hc.tp[req, j]))
                mism[j] = (tp16 != np.float32(rowmax))
                ci = int(q4[qt, 1])
                wpos = int(q4[qt, 2])
                base, cc, half = _WINMAP[ci]
                m, wr = wpos // WIN, wpos % WIN
                tok[j] = qt * QW + base + cc * WIN + m * half + wr
            first_mm = int(mism[:npr].argmax()) if mism[:npr].any() else npr
            copy_len = min(first_mm + 1, npr)
            row = out[req]
            for j in range(copy_len):
                row[j] = tok[j] if j == first_mm else hc.dtok[req, j]
            if first_mm >= npr:
                row[npr] = bonus[req]
    return out


# ---------------- shape-agnostic fallback (host compute) ----------------

def _kernel_numpy(output_token_ids, cu_num_draft_tokens, draft_token_ids,
                  draft_probs, target_probs, bonus_token_ids, uniform_probs,
                  q, is_greedy):
    out = np.array(output_token_ids, dtype=np.int32).copy()
    Bb, Sp1 = out.shape
    Sl = Sp1 - 1
    Nt = draft_token_ids.shape[0]
    cu = np.asarray(cu_num_draft_tokens, dtype=np.int64)
    n_per = np.diff(np.concatenate([[0], cu]))
    start_ = cu - n_per
    tam = target_probs.argmax(axis=-1).astype(np.int32)
    prob = np.maximum(target_probs - draft_probs, 0.0)
    req_id = np.searchsorted(cu, np.arange(Nt), side="right")
    rec = (prob / q[req_id]).argmax(axis=1).astype(np.int32)
    for r in range(Bb):
        npr = int(n_per[r]); st = int(start_[r])
        if is_greedy[r]:
            k = npr
            for j in range(npr):
                g = min(st + j, Nt - 1)
                if draft_token_ids[g] != tam[g]:
                    k = j
                    break
            for j in range(min(k + 1, npr)):
                out[r, j] = tam[min(st + j, Nt - 1)]
            if k >= npr and npr < Sp1:
                out[r, npr] = bonus_token_ids[r]
        else:
            fr = Sl
            for j in range(npr):
                g = min(st + j, Nt - 1)
                dpv = draft_probs[g, draft_token_ids[g]]
                tpv = target_probs[g, draft_token_ids[g]]
                ok = dpv > 0 and (tpv / dpv) >= uniform_probs[g]
                if not ok:
                    fr = j
                    break
            for j in range(npr):
                g = min(st + j, Nt - 1)
                if j < fr:
                    out[r, j] = draft_token_ids[g]
                elif j == fr:
                    out[r, j] = rec[g]
                else:
                    break
            if fr >= npr and npr < Sp1:
                out[r, npr] = bonus_token_ids[r]
    return out


def _shapes_ok(inputs):
    try:
        return (inputs["output_token_ids"].shape == (64, 9)
                and inputs["cu_num_draft_tokens"].shape == (64,)
                and inputs["draft_token_ids"].shape == (512,)
                and inputs["draft_probs"].shape == (512, 32000)
                and inputs["target_probs"].shape == (512, 32000)
                and inputs["bonus_token_ids"].shape == (64,)
                and inputs["uniform_probs"].shape == (512,)
                and inputs["q"].shape == (64, 32000)
                and inputs["is_greedy"].shape == (64,))
    except Exception:
        return False


_CACHE = {}


def _get_nc(kind):
    if kind not in _CACHE:
        _CACHE[kind] = build_kernel_fast(n_devices=8)
    return _CACHE[kind]


def kernel(**inputs):
    inputs = {k: np.asarray(v) for k, v in inputs.items()}
    if not _shapes_ok(inputs):
        return _kernel_numpy(**inputs)
    perm = plan_permutation(inputs)
    if perm is not None:
        nc = _get_nc("fast")
        hc = host_precompute(inputs, perm)
        in_maps = shard_inputs_fast(inputs, perm, hc=hc)
        res = bass_utils.run_bass_kernel_spmd(nc, in_maps,
                                              core_ids=list(range(8)))
        return assemble_outputs_fast(res.results, perm, inputs, hc)
    return _kernel_numpy(**inputs)


# revision 11
# speedup vs baseline: 1.1331x; 1.1331x over previous
"""AscendRejectionSampler — Trainium2 Bass kernel (8-core SPMD), v3.5.

Work split (device does all O(rows x V) arithmetic, host does O(N)
scalar logic + gathers + dtype packing + final assembly):

- Host computes the accept/reject chain (dp/tp/u are O(N) gathers, the
  same class as the uniform_probs/draft-token packing v2 already did)
  -> first_rej per request, so each non-greedy request needs exactly
  ONE (t, d) row on device: argmax((t-d)/q).
- Greedy requests need row-max of each target row (mismatch test
  happens on host by comparing fp16(tp) vs the returned row-max) plus
  the argmax *position* per row.
- Device inputs are fp16 (t, d rows) / bf16 (q rows): ~2.75 MB per
  core instead of 5.7 MB.  Device returns a [128, 5] f32 tile of
  per-partition (max, argmax-window, window-pos) data; host decodes
  and assembles the [64, 9] output.

Device program per core (4 greedy + 4 non-greedy requests):
- tg [128, 8000] f16: 32 greedy target rows x 4 quarters, loaded as 5
  uneven chunks [1000,2000,2000,2000,1000] alternating the two HWDGE
  rings (small first chunk starts DVE early, small last chunk keeps
  the final fold+reduce short; 2-4KB lines are the DMA sweet spot).
- per chunk: one pairwise-max fold (fp16 tensor_tensor runs 2
  elem/cycle) + 125-wide tensor_reduce -> dense cmax [128, 32].
- max8/max_index on cmax -> winning window ci; ONE indirect gather of
  row p*32+ci from tgp, a host-side pair-interleaved copy of tg where
  each window's two fold halves are a contiguous 250-elem row;
  max_index on the window gives the position.  (tgp costs DRAM
  footprint but is only ever read 250 elems/partition.)
- ng [128, 2000] f16 (t|d of the 4 winning rows, 32 segs each) and
  qn [128, 1000] bf16: rq = Exp(-Ln(q)) runs on the otherwise-idle
  ACT engine (Reciprocal table is gated off), diff and ratio are fp16
  2x tensor_tensor ops, max8 + max_index -> seg max and position.
- gpsimd tensor ops are avoided entirely: they stall DVE ~4x via SBUF
  port contention (measured).  tensor_tensor_reduce crashes this
  runtime (unsupported) — plain ops only.
"""
import sys
sys.path.insert(0, '/opt/trn_rl_repo')
import numpy as np
import concourse.bass as bass
import concourse.bacc as bacc
import concourse.tile as tile
from concourse import mybir
from concourse import bass_utils
from concourse.tile import add_dep_helper

f32 = mybir.dt.float32
i32 = mybir.dt.int32
u32 = mybir.dt.uint32
f16 = mybir.dt.float16
bf16 = mybir.dt.bfloat16
Alu = mybir.AluOpType
AX = mybir.AxisListType

R = 8             # requests per core
S = 8
SP1 = 9
V = 32000
B = 64
NG = 4            # slots 0-3 non-greedy, 4-7 greedy
QW = 8000         # quarter width (vocab span per greedy partition)
CW = 2000         # chunk width (per greedy partition, 4 chunks)
SEGW = 1000       # ng segment width (32 segs per row)
WIN = 125         # window width
F16_NINF = -65504.0
CHUNK_WS = [1000, 1500, 2000, 2000, 1500]
CHUNK_BASES = [0, 1000, 2500, 4500, 6500]
# window ci -> (chunk base, within-chunk 125-block, fold-partner stride)
_WINMAP = []
for _b, _w in zip(CHUNK_BASES, CHUNK_WS):
    for _c in range(_w // 250):
        _WINMAP.append((_b, _c, _w // 2))


def build_kernel_fast(n_devices=8):
    nc = bacc.Bacc("TRN2", target_bir_lowering=False, debug=False,
                   enable_asserts=False, num_devices=n_devices)
    tg = nc.dram_tensor("tg", [128, QW], f16, kind="ExternalInput").ap()
    tgp = nc.dram_tensor("tgp", [128, 32 * 256], f16,
                         kind="ExternalInput").ap()
    ngt = nc.dram_tensor("ngt", [128, 2 * SEGW], f16,
                         kind="ExternalInput").ap()
    qn = nc.dram_tensor("qn", [128, SEGW], bf16, kind="ExternalInput").ap()
    out = nc.dram_tensor("out", [128, 5], f32, kind="ExternalOutput").ap()
    with tile.TileContext(nc) as tc:
        _body(tc, nc, tg, tgp, ngt, qn, out)
    nc.compile()
    return nc


def _body(tc, nc, tg, tgp, ngt, qn, out):
    import contextlib
    ctx = contextlib.ExitStack()
    with ctx:
        small = ctx.enter_context(tc.tile_pool(name="small", bufs=1))
        big = ctx.enter_context(tc.tile_pool(name="big", bufs=1))

        V_ = nc.vector
        G_ = nc.gpsimd
        A_ = nc.scalar

        def tt(out_, a, b, op):
            return V_.tensor_tensor(out_, a, b, op=op)

        # ---------------- DMAs: greedy chunks first, uneven widths --
        # small first chunk starts DVE early, small last chunk keeps the
        # final fold+reduce short; ng/q data behind them.
        # widths in elements; each W must be a multiple of 250.
        WS = [1000, 1500, 2000, 2000, 1500]
        BASES = [0, 1000, 2500, 4500, 6500]
        NCH = len(WS)
        ch = [big.tile([128, WS[k]], f16, name=f"tg{k}")
              for k in range(NCH)]
        qtile = big.tile([128, SEGW], bf16)
        ngtile = big.tile([128, 2 * SEGW], f16)
        ring = [nc.sync, nc.scalar, nc.sync, nc.scalar, nc.sync]
        for k in range(NCH):
            ring[k].dma_start(ch[k][:], tg[:, BASES[k]:BASES[k] + WS[k]])
        # q right after c3 on the scalar ring: early enough that the
        # ACT Ln/Exp reciprocal beats the ng multiply, late enough not
        # to displace any greedy chunk
        nc.scalar.dma_start(qtile[:], qn[:, :])
        nc.scalar.dma_start(ngtile[:], ngt[:, :])

        # gpsimd: partition iota (p*32) for the window-gather offsets
        iota32 = small.tile([128, 1], i32)
        G_.iota(iota32[:], pattern=[[0, 1]], base=0, channel_multiplier=32)

        # ---------------- greedy: per-chunk fp16 pairwise-max fold (2x)
        # + 125-wide reduce into a dense cmax[128,32]; chunk k's W/250
        # entries cover x{base_k + 125c + (W_k/2)m}, m in {0,1}.
        cmax = small.tile([128, 32], f16)
        l1 = big.tile([128, QW // 2], f16)
        prev = None
        lb = 0
        cb = 0
        for k in range(NCH):
            h = WS[k] // 2
            fi = tt(l1[:, lb:lb + h],
                    ch[k][:, 0:h], ch[k][:, h:2 * h], Alu.max)
            if prev is not None:
                add_dep_helper(fi.ins, prev.ins, sync=False,
                               reason="DVE: consume chunks in arrival order")
            nwin = WS[k] // 250
            prev = V_.tensor_reduce(
                cmax[:, cb:cb + nwin],
                l1[:, lb:lb + h].rearrange("p (b c) -> p b c", c=WIN),
                axis=AX.X, op=Alu.max)
            lb += h
            cb += nwin

        cm8 = small.tile([128, 8], f16)
        V_.max(out=cm8[:], in_=cmax[:])
        ci8 = small.tile([128, 8], u32)
        cii = V_.max_index(out=ci8[:], in_max=cm8[:], in_values=cmax[:])

        # ONE window gather: tgp is the host-side 125-interleaved pair
        # layout, where window ci's two fold halves are the contiguous
        # 250-wide row p*32 + ci of the [128*32, 250] view.
        # offs add on gpsimd: the gather emission starts straight off
        # the ci semaphore instead of queueing behind DVE ops ([128,1]
        # op -- no measurable SBUF port contention at this size)
        offs = small.tile([128, 1], i32)
        o2i = G_.tensor_tensor(offs[:], iota32[:],
                               ci8.bitcast(i32)[:, 0:1], op=Alu.add)
        win = big.tile([128, 256], f16)
        tgv = tgp.rearrange("p (g w) -> (p g) w", w=256)
        G_.indirect_dma_start(
            out=win[:], out_offset=None, in_=tgv,
            in_offset=bass.IndirectOffsetOnAxis(ap=offs[:], axis=0))
        wi8 = small.tile([128, 8], u32)
        V_.max_index(out=wi8[:], in_max=cm8[:], in_values=win[:])

        # ---------------- non-greedy: ratio = (t - d) * (1/q), argmax.
        # 1/q = Exp(-Ln(q)) runs entirely on the otherwise-idle ACT
        # engine (the Reciprocal table is gated off); bf16 rq keeps the
        # ratio multiply in the fp16 2x DVE mode.
        lq = big.tile([128, SEGW], f32)
        A_.activation(lq[:], qtile[:], mybir.ActivationFunctionType.Ln)
        rq = big.tile([128, SEGW], bf16)
        A_.activation(rq[:], lq[:], mybir.ActivationFunctionType.Exp,
                      scale=-1.0)
        diff = big.tile([128, SEGW], f16)
        di = tt(diff[:], ngtile[:, 0:SEGW], ngtile[:, SEGW:2 * SEGW],
                Alu.subtract)
        # keep the gather-offset chain ahead of the ng chain on DVE
        add_dep_helper(di.ins, cii.ins, sync=False,
                       reason="DVE: ci find before ng work")
        ratio = big.tile([128, SEGW], f16)
        tt(ratio[:], diff[:], rq[:], Alu.mult)
        # two fp16 2x folds: scan only [128,250]; the 4-way position
        # ambiguity is resolved host-side with the exact f32 formula
        rf1 = big.tile([128, SEGW // 2], f16)
        tt(rf1[:], ratio[:, 0:500], ratio[:, 500:1000], Alu.max)
        rf2 = big.tile([128, SEGW // 4], f16)
        tt(rf2[:], rf1[:, 0:250], rf1[:, 250:500], Alu.max)
        n8 = small.tile([128, 8], f16)
        V_.max(out=n8[:], in_=rf2[:])
        ni8 = small.tile([128, 8], u32)
        V_.max_index(out=ni8[:], in_max=n8[:], in_values=rf2[:])

        # ---------------- output pack [128,5]: qmax, ci, wpos, segmax, npos
        # (DVE casts -- keeping ACT on the Ln/Exp table set)
        outp = small.tile([128, 5], f32)
        V_.tensor_copy(out=outp[:, 0:1], in_=cm8[:, 0:1])
        V_.tensor_copy(out=outp[:, 1:2], in_=ci8[:, 0:1])
        V_.tensor_copy(out=outp[:, 2:3], in_=wi8[:, 0:1])
        V_.tensor_copy(out=outp[:, 3:4], in_=n8[:, 0:1])
        V_.tensor_copy(out=outp[:, 4:5], in_=ni8[:, 0:1])
        nc.sync.dma_start(out[:, :], outp[:], single_packet=True)


# ---------------- host side ----------------

def plan_permutation(inputs, n_cores=8):
    """Slot assignment: 4 non-greedy (slots 0-3) + 4 greedy (slots 4-7)."""
    isg = np.asarray(inputs["is_greedy"]).astype(bool)
    Bb = isg.shape[0]
    if Bb != B or Bb // n_cores != R:
        return None
    g = np.where(isg)[0]
    n = np.where(~isg)[0]
    if len(g) != Bb // 2:
        return None
    perm = np.empty(Bb, np.int64)
    for c in range(n_cores):
        perm[c * R:c * R + NG] = n[c * NG:(c + 1) * NG]
        perm[c * R + NG:(c + 1) * R] = g[c * NG:(c + 1) * NG]
    return perm


class _HostCtx:
    pass


def host_precompute(inputs, perm):
    """The O(N) scalar chain: accept/reject, first_rej, winning rows."""
    hc = _HostCtx()
    cu = inputs["cu_num_draft_tokens"].astype(np.int64)
    N = inputs["draft_token_ids"].shape[0]
    n_per = np.diff(np.concatenate([[0], cu]))
    start = cu - n_per
    gidx = np.clip(start[:, None] + np.arange(S)[None, :], 0, N - 1)  # [B,S]
    dtok = inputs["draft_token_ids"][gidx].astype(np.int64)           # [B,S]
    dp = inputs["draft_probs"][gidx, dtok]                            # [B,S]
    tp = inputs["target_probs"][gidx, dtok]
    u = inputs["uniform_probs"][gidx]
    jj = np.arange(S)[None, :]
    valid = jj < n_per[:, None]
    acc = (dp > 0.0) & ((tp / dp) >= u)
    rej = (~acc) & valid
    first_rej = np.where(rej.any(axis=1), rej.argmax(axis=1), S)      # [B]
    win_j = np.minimum(first_rej, np.maximum(n_per - 1, 0))
    win_row = gidx[np.arange(B), win_j]                               # [B]
    hc.n_per, hc.gidx, hc.dtok, hc.tp = n_per, gidx, dtok, tp
    hc.first_rej, hc.win_row, hc.perm = first_rej, win_row, perm
    return hc


def shard_inputs_fast(inputs, perm, n_cores=8, hc=None):
    import ml_dtypes
    if hc is None:
        hc = host_precompute(inputs, perm)
    tprobs = inputs["target_probs"]
    dprobs = inputs["draft_probs"]
    qarr = inputs["q"]
    in_maps = []
    for c in range(n_cores):
        reqs = perm[c * R:(c + 1) * R]
        # non-greedy slots 0-3: one winning row each, 32 segs
        ngp = np.empty((128, 2 * SEGW), np.float16)
        qp = np.empty((128, SEGW), ml_dtypes.bfloat16)
        for s in range(NG):
            req = reqs[s]
            w = hc.win_row[req]
            ngp[s * 32:(s + 1) * 32, 0:SEGW] = \
                tprobs[w].reshape(32, SEGW).astype(np.float16)
            ngp[s * 32:(s + 1) * 32, SEGW:] = \
                dprobs[w].reshape(32, SEGW).astype(np.float16)
            qp[s * 32:(s + 1) * 32] = \
                qarr[req].reshape(32, SEGW).astype(ml_dtypes.bfloat16)
        # greedy slots 4-7: 8 target rows each, 4 quarters per row
        tgq = np.empty((128, QW), np.float16)
        for t in range(NG):
            req = reqs[NG + t]
            rows = hc.gidx[req]
            tgq[t * 32:(t + 1) * 32] = \
                tprobs[rows].astype(np.float16).reshape(32, QW)
        # pair-interleaved copy: window ci's two fold halves contiguous
        tgpair = np.empty_like(tgq)
        for b, w in zip(CHUNK_BASES, CHUNK_WS):
            blk = tgq[:, b:b + w].reshape(128, 2, w // 250, WIN)
            tgpair[:, b:b + w] = np.ascontiguousarray(
                blk.transpose(0, 2, 1, 3)).reshape(128, w)
        # rows padded 250 -> 256 elems (512B gather descriptors)
        tgpad = np.zeros((128, 32, 256), np.float16)
        tgpad[:, :, 0:250] = tgpair.reshape(128, 32, 250)
        in_maps.append(dict(tg=tgq, tgp=tgpad.reshape(128, 32 * 256),
                            ngt=ngp, qn=qp))
    return in_maps


def assemble_outputs_fast(results, perm, inputs, hc):
    out = np.array(inputs["output_token_ids"], dtype=np.int32).copy()
    bonus = inputs["bonus_token_ids"]
    for c in range(len(results)):
        o = np.asarray(results[c]["out"], dtype=np.float64)  # [128,5]
        reqs = perm[c * R:(c + 1) * R]
        for s in range(NG):                    # non-greedy
            req = reqs[s]
            npr = int(hc.n_per[req])
            fr = int(hc.first_rej[req])
            row = out[req]
            for j in range(min(fr, npr)):
                row[j] = hc.dtok[req, j]
            if fr < npr:
                segs = o[s * 32:(s + 1) * 32]
                vals = segs[:, 3]
                seg = int(vals.argmax())       # first max (vocab order)
                ni = int(segs[seg, 4])
                w = hc.win_row[req]
                best, bestv = 0, -np.inf
                for m in range(4):
                    pos = seg * SEGW + ni + 250 * m
                    pr = max(
                        float(inputs["target_probs"][w, pos])
                        - float(inputs["draft_probs"][w, pos]), 0.0
                    ) / float(inputs["q"][req, pos])
                    if pr > bestv:
                        bestv, best = pr, pos
                row[fr] = best
            elif npr < SP1:
                row[npr] = bonus[req]
        for t in range(NG):                    # greedy
            req = reqs[NG + t]
            npr = int(hc.n_per[req])
            tok = np.empty(S, np.int64)
            mism = np.zeros(S, bool)
            for j in range(npr):
                q4 = o[(t * 8 + j) * 4:(t * 8 + j) * 4 + 4]
                qmax = q4[:, 0]
                qt = int(qmax.argmax())        # first quarter with max
                rowmax = qmax[qt]
                tp16 = np.float32(np.float16(hc.tp[req, j]))
                mism[j] = (tp16 != np.float32(rowmax))
                ci = int(q4[qt, 1])
                wpos = int(q4[qt, 2])
                base, cc, half = _WINMAP[ci]
                m, wr = wpos // WIN, wpos % WIN
                tok[j] = qt * QW + base + cc * WIN + m * half + wr
            first_mm = int(mism[:npr].argmax()) if mism[:npr].any() else npr
            copy_len = min(first_mm + 1, npr)
            row = out[req]
            for j in range(copy_len):
                row[j] = tok[j] if j == first_mm else hc.dtok[req, j]
            if first_mm >= npr:
                row[npr] = bonus[req]
    return out


# ---------------- shape-agnostic fallback (host compute) ----------------

def _kernel_numpy(output_token_ids, cu_num_draft_tokens, draft_token_ids,
                  draft_probs, target_probs, bonus_token_ids, uniform_probs,
                  q, is_greedy):
    out = np.array(output_token_ids, dtype=np.int32).copy()
    Bb, Sp1 = out.shape
    Sl = Sp1 - 1
    Nt = draft_token_ids.shape[0]
    cu = np.asarray(cu_num_draft_tokens, dtype=np.int64)
    n_per = np.diff(np.concatenate([[0], cu]))
    start_ = cu - n_per
    tam = target_probs.argmax(axis=-1).astype(np.int32)
    prob = np.maximum(target_probs - draft_probs, 0.0)
    req_id = np.searchsorted(cu, np.arange(Nt), side="right")
    rec = (prob / q[req_id]).argmax(axis=1).astype(np.int32)
    for r in range(Bb):
        npr = int(n_per[r]); st = int(start_[r])
        if is_greedy[r]:
            k = npr
            for j in range(npr):
                g = min(st + j, Nt - 1)
                if draft_token_ids[g] != tam[g]:
                    k = j
                    break
            for j in range(min(k + 1, npr)):
                out[r, j] = tam[min(st + j, Nt - 1)]
            if k >= npr and npr < Sp1:
                out[r, npr] = bonus_token_ids[r]
        else:
            fr = Sl
            for j in range(npr):
                g = min(st + j, Nt - 1)
                dpv = draft_probs[g, draft_token_ids[g]]
                tpv = target_probs[g, draft_token_ids[g]]
                ok = dpv > 0 and (tpv / dpv) >= uniform_probs[g]
                if not ok:
                    fr = j
                    break
            for j in range(npr):
                g = min(st + j, Nt - 1)
                if j < fr:
                    out[r, j] = draft_token_ids[g]
                elif j == fr:
                    out[r, j] = rec[g]
                else:
                    break
            if fr >= npr and npr < Sp1:
                out[r, npr] = bonus_token_ids[r]
    return out


def _shapes_ok(inputs):
    try:
        return (inputs["output_token_ids"].shape == (64, 9)
                and inputs["cu_num_draft_tokens"].shape == (64,)
                and inputs["draft_token_ids"].shape == (512,)
                and inputs["draft_probs"].shape == (512, 32000)
                and inputs["target_probs"].shape == (512, 32000)
                and inputs["bonus_token_ids"].shape == (64,)
                and inputs["uniform_probs"].shape == (512,)
                and inputs["q"].shape == (64, 32000)
                and inputs["is_greedy"].shape == (64,))
    except Exception:
        return False


_CACHE = {}


def _get_nc(kind):
    if kind not in _CACHE:
        _CACHE[kind] = build_kernel_fast(n_devices=8)
    return _CACHE[kind]


def kernel(**inputs):
    inputs = {k: np.asarray(v) for k, v in inputs.items()}
    if not _shapes_ok(inputs):
        return _kernel_numpy(**inputs)
    perm = plan_permutation(inputs)
    if perm is not None:
        nc = _get_nc("fast")
        hc = host_precompute(inputs, perm)
        in_maps = shard_inputs_fast(inputs, perm, hc=hc)
        res = bass_utils.run_bass_kernel_spmd(nc, in_maps,
                                              core_ids=list(range(8)))
        return assemble_outputs_fast(res.results, perm, inputs, hc)
    return _kernel_numpy(**inputs)
